# revision 89
# baseline (speedup 1.0000x reference)
"""Trainium2 Bass kernel for nn_LCAMatrixModel (pairwise selu-MLP scoring).

o[i,j] = hardsigmoid( sum_h W2b[h]*selu(g[i,h]+g[j,h]+b2a[h]) + b2b )
with g = f(x) a small per-node MLP chain. o is symmetric.

Decomposition (u = g_i+g_j+b2a, m = min(u,0)):
  sum_h w*selu(u) = lam*(c_i+c_j+K0) - lam*al*SW
                    + sum_h (lam*w)*al*e^m + sum_h (-lam*w)*m
with c_i = sum_h w*g[i,h] rank-1 (PE close via ones64 x c_row + Bcol bias).
Only the e-part (al*e^m) and m-part need the full N^2*H elementwise work:
  m-part: min(u,0) scaled by ratio = w/fp8(w) (folds the fp8 stationary
          quantization into the data), written as fp8; reduced by ONE fp8
          DoubleRow matmul per (pair-batch u, chunk) covering 4 output rows
          (2 pairs x 2 rows) at 0.5 cycles/row. 12 of 16 u's compute -m on
          ACT (relu(-u), sign folded into the stationary), 4 on DVE.
  e-part: al*e^m = min(P_i*P_j, al) with P = sqrt(al)*e^(g+b2a/2), computed
          on DVE (tensor_scalar mult+min, per-pair P_i scalar read straight
          from a col-shifted f32 tile - no gather DMAs); kept bf16 and
          reduced by per-pair bf16 matmuls (fp8 here fails the error gate:
          both its saturated and mid-range quantization alone exceed it).
Other tricks: encoder folded into nn1 on the host (W1a@W_enc); selu's
-lam*al constant folded into downstream biases (one fused DVE
scalar_tensor_tensor per selu); pairs trimmed below the local diagonal with
a monotone per-u trim (early pairs initialize all psum regions); preamble
chunks emitted in order 2,1,0 interleaved into the pairwise groups so each
engine's instruction stream (= its schedule) overlaps preamble and main
work; matmuls grouped by stationary so weight reloads can be elided.

Sharding: np.roll(x, -c) per core -> core c owns global rows {c, c+8, ...};
each core computes its local upper triangle (output is symmetric); host
mirrors. Output DMA'd as bf16, host converts to f32.
"""
import sys

sys.path.insert(0, "/opt/trn_rl_repo")

import numpy as np
import ml_dtypes

N_NODES = 1536
RAW = 512
D = 128
H = 64
NCORES = 8
ROWS = N_NODES // NCORES  # own rows per core = 192
PAIRS = ROWS // 2         # 96
GROUPS = PAIRS // 32      # 3 groups of 32 pairs (64 out rows each)
NCHUNK = 3                # 512-wide j chunks
CW = 512

LAM = 1.0507009873554805
AL = 1.6732632423543772

# ---- route tables (per u in 0..15, same for each group), tuned against
# probed HW rates: DVE ts dual-op ~0.74 ns/col, ACT ~1.23, Pool useless,
# PE fp8 DoubleRow ~0.4 ns/col + ldweights ----
# m-op engine: ACT computes -m = relu(-u) (so downstream flips signs).
# e' comes from the DVE P-product route for every u (exp of fp8 m would
# amplify quantization); ACT's only main-loop work is these m ops.
M_ACT = {4, 5, 6, 7, 8, 9, 10, 11, 12, 13, 14, 15}

_compiled = None


def _build_program():
    import concourse.bacc as bacc
    import concourse.mybir as mybir
    import concourse.tile as tile

    F32 = mybir.dt.float32
    BF16 = mybir.dt.bfloat16
    FP8 = mybir.dt.float8e4
    AF = mybir.ActivationFunctionType
    OP = mybir.AluOpType
    DR = mybir.MatmulPerfMode.DoubleRow

    nc = bacc.Bacc("TRN2", target_bir_lowering=False, debug=False)

    # ---- DRAM I/O ----
    xT_d = nc.dram_tensor("xT", [RAW, N_NODES], BF16, kind="ExternalInput")
    # wpack: wcT (4 k-slices of (W1a@W_enc).T), w1bT (rows 0:64), w2aT  [128, 448]
    wpack_d = nc.dram_tensor("wpack", [D, 4 * H + D + H], BF16, kind="ExternalInput")
    wlb_d = nc.dram_tensor("wlb", [H, 1], BF16, kind="ExternalInput")
    # cpack columns: 0 bc, 1 bcl, 2 b1b, 3 b1bl, 4 gbias, 5 ratio2,
    # 6 cfin, 7 ratio2n, 8 pbias, 9 alrat
    cpack_d = nc.dram_tensor("cpack", [D, 10], F32, kind="ExternalInput")
    # e-part stationaries: per pair-slot [128, 64] bf16, 32 slots
    w32e_d = nc.dram_tensor("w32e", [D, 32 * H], BF16, kind="ExternalInput")
    # m-part DoubleRow stationaries: per u-slot [128, 2, 64] fp8, 16 slots
    w32m_d = nc.dram_tensor("w32m", [D, 16 * 2 * H], FP8, kind="ExternalInput")
    ones64_d = nc.dram_tensor("ones64", [1, H], BF16, kind="ExternalInput")
    out_d = nc.dram_tensor("out", [ROWS, N_NODES], BF16, kind="ExternalOutput")

    with tile.TileContext(nc) as tc:
        with (
            tc.tile_pool(name="cst", bufs=1) as cst,
            tc.tile_pool(name="pre", bufs=2) as pre,
            tc.tile_pool(name="mp", bufs=6) as mp,
            tc.tile_pool(name="ep", bufs=6) as ep,
            tc.tile_pool(name="op", bufs=4) as opool,
            tc.tile_pool(name="ps", bufs=2, space="PSUM") as ps,
            tc.tile_pool(name="psm", bufs=6, space="PSUM") as psm,
        ):
            # ---- load constants (scalar + gpsimd queues; sync queue is
            # reserved for the x chunks so compute can start ASAP) ----
            wpack = cst.tile([D, 4 * H + D + H], BF16)
            nc.scalar.dma_start(wpack[:], wpack_d[:])
            wlb = cst.tile([H, 1], BF16)
            nc.scalar.dma_start(wlb[:], wlb_d[:])
            wcT = wpack[:, 0 : 4 * H]
            w1bT = wpack[0:H, 4 * H : 4 * H + D]
            w2aT = wpack[:, 4 * H + D : 4 * H + D + H]
            cpack = cst.tile([D, 10], F32)
            nc.scalar.dma_start(cpack[:], cpack_d[:])
            bc = cpack[0:H, 0:1]
            bcl = cpack[0:H, 1:2]
            b1b = cpack[:, 2:3]
            b1bl = cpack[:, 3:4]
            gbias = cpack[0:H, 4:5]
            ratio2 = cpack[:, 5:6]
            cfin = cpack[:, 6:7]
            pbias = cpack[0:H, 8:9]
            alrat = cpack[:, 9:10]
            ones64 = cst.tile([1, H], BF16)
            nc.scalar.dma_start(ones64[:], ones64_d[:])
            w32e = cst.tile([D, 32 * H], BF16)
            nc.gpsimd.dma_start(w32e[:], w32e_d[:])
            w32m = cst.tile([D, 16 * 2 * H], FP8)
            nc.gpsimd.dma_start(w32m[:], w32m_d[:])
            w32m_s = w32m[:, :].rearrange("p (s i m) -> p s i m", i=2, m=H)

            F32R = mybir.dt.float32r

            # ---- selu chain helper: out = selu(psum + b) + lam*al ----
            # (the +lam*al constant is folded into the next layer's bias)
            def selu_from_psum(out_ap, pa, b_raw, b_lam, p):
                r = pre.tile([p, CW], F32, tag="selr")
                nc.scalar.activation(r[:], pa, AF.Relu, bias=b_lam, scale=LAM)
                m = pre.tile([p, CW], F32, tag="selm")
                nc.vector.tensor_scalar(m[:], pa, b_raw, 0.0, OP.add, OP.min)
                e = pre.tile([p, CW], F32, tag="sele")
                nc.scalar.activation(e[:], m[:], AF.Exp)
                nc.vector.scalar_tensor_tensor(
                    out_ap, e[:], LAM * AL, r[:], OP.mult, OP.add
                )

            # ---- per-chunk preamble (chunk order 2,1,0 so group G=2 can
            # start its pairwise work while chunks 1,0 are still cooking) ----
            xt = cst.tile([D, 4 * N_NODES], BF16)
            a1T = cst.tile([H, N_NODES], BF16)
            hT = cst.tile([D, N_NODES], BF16)
            # g2c = g + b2a/2 - delta_g (selu-const compensation), f32 [64, N]
            g2c = cst.tile([H, N_NODES], F32)
            g2b = cst.tile([D, N_NODES], BF16)  # both halves = g2c (bf16)
            # per-pair scalar sources: col 16t = [g2c_i1 ; g2c_i2] (bottom
            # half shifted by 8 cols)
            g2s = cst.tile([D, N_NODES], F32)
            # ratio-scaled m-operands (w-quantization folded into the data):
            # g2bm = ratio*g2c both halves, g2sm/g2smn = +-ratio*g2s
            g2bm = cst.tile([D, N_NODES], BF16)
            g2sm = cst.tile([D, N_NODES], F32)
            g2smn = cst.tile([D, N_NODES], F32)
            # P-route tensors: P = sqrt(al*eratio)*exp(g2c); P2 bf16 both
            # halves, Ps f32 scalar source (bottom shifted by 8, like g2s)
            P2 = cst.tile([D, N_NODES], BF16)
            Ps = cst.tile([D, N_NODES], F32)
            gown = cst.tile([H, ROWS], BF16)
            c_row = cst.tile([1, N_NODES], BF16)
            Bcol = cst.tile([H, GROUPS], F32)

            g2b_own = g2b[0:H, :].rearrange("p (a b) -> p a b", b=8)

            # the shifted bottom halves leave the last 8 cols unwritten;
            # they are never read as scalars but full-width copies of them are
            nc.vector.memset(g2s[H:D, N_NODES - 8 : N_NODES], 0.0)

            def preamble_part1(c):
                cs = slice(c * CW, (c + 1) * CW)
                for k in range(4):
                    nc.sync.dma_start(
                        xt[:, k * N_NODES + c * CW : k * N_NODES + (c + 1) * CW],
                        xT_d[k * D : (k + 1) * D, cs],
                    )
                # a1 = selu(W_combo @ x + b_combo)   (encoder folded in)
                pa = ps.tile([H, CW], F32, tag="ps")
                for k in range(4):
                    nc.tensor.matmul(
                        pa[:],
                        wcT[:, k * H : (k + 1) * H],
                        xt[:, k * N_NODES + c * CW : k * N_NODES + (c + 1) * CW],
                        start=(k == 0),
                        stop=(k == 3),
                    )
                selu_from_psum(a1T[:, cs], pa[:], bc[:, 0:1], bcl[:, 0:1], H)

            def preamble_part2(c):
                cs = slice(c * CW, (c + 1) * CW)
                # h = selu(W1b @ a1 + b1b)
                ph = ps.tile([D, CW], F32, tag="ps")
                nc.tensor.matmul(
                    ph[:], w1bT[:], a1T[:, cs], start=True, stop=True,
                )
                selu_from_psum(hT[:, cs], ph[:], b1b[:, 0:1], b1bl[:, 0:1], D)
                # g = W2a @ h
                pg = ps.tile([H, CW], F32, tag="ps")
                nc.tensor.matmul(
                    pg[:], w2aT[:], hT[:, cs], start=True, stop=True,
                )
                # g2c = g + b2a/2 - delta_g (bias-compensated), from psum
                nc.scalar.activation(g2c[:, cs], pg[:], AF.Identity, bias=gbias)
                # g2b both halves = g2c (bf16); DVE casts the top, the idle
                # SP/DMA path replicates bottom halves (same dtype, no cast)
                nc.vector.tensor_copy(g2b[0:H, cs], g2c[:, cs])
                nc.sync.dma_start(g2b[H:D, cs], g2b[0:H, cs])
                # scalar-source tiles: g2s top = g2c, bottom = g2c shifted by
                # 8 cols (so col 16t = [g_i1; g_i2] for pair t).
                # Chunk order 2,1,0 means cols (c+1)*CW..+8 already exist.
                nc.vector.tensor_copy(g2s[0:H, cs], g2c[:, cs])
                wsh = CW if c < NCHUNK - 1 else CW - 8
                nc.sync.dma_start(
                    g2s[H:D, c * CW : c * CW + wsh],
                    g2c[:, c * CW + 8 : c * CW + 8 + wsh],
                )
                # ratio-scaled m-operand tiles
                nc.vector.tensor_scalar(
                    g2bm[0:H, cs], g2c[:, cs], ratio2[0:H, 0:1], None, OP.mult
                )
                nc.sync.dma_start(g2bm[H:D, cs], g2bm[0:H, cs])
                nc.vector.tensor_scalar(
                    g2sm[:, cs], g2s[:, cs], ratio2[:, 0:1], None, OP.mult
                )
                nc.scalar.activation(
                    g2smn[:, cs], g2sm[:, cs], AF.Identity, scale=-1.0
                )
                # P-route tensors: Ps top f32 = sqrt(al*eratio)*exp(g2c),
                # bottom shifted by 8; P2 = Ps in bf16 (both halves aligned)
                nc.scalar.activation(
                    Ps[0:H, cs], g2c[:, cs], AF.Exp, bias=pbias
                )
                nc.vector.tensor_copy(P2[0:H, cs], Ps[0:H, cs])
                nc.sync.dma_start(P2[H:D, cs], P2[0:H, cs])
                nc.sync.dma_start(
                    Ps[H:D, c * CW : c * CW + wsh],
                    Ps[0:H, c * CW + 8 : c * CW + 8 + wsh],
                )
                # c_row chunk = wl.T @ g2b (includes lam; K0/2 shift folded
                # into CONST on the host)
                pc = ps.tile([1, CW], F32, tag="ps")
                nc.tensor.matmul(
                    pc[:], wlb[:], g2b[0:H, cs], start=True, stop=True,
                )
                nc.scalar.activation(c_row[0:1, cs], pc[:], AF.Copy)
                # group-G own-col gather (for Bcol only; off critical path)
                G = c
                osl = slice(H * G, H * (G + 1))
                nc.sync.dma_start(gown[:, osl], g2b_own[:, osl, 0:1])
                pb = ps.tile([H, 1], F32, tag="ps", name=f"pb_{G}")
                nc.tensor.matmul(
                    pb[:], gown[:, osl], wlb[:], start=True, stop=True,
                )
                nc.scalar.activation(
                    Bcol[:, G : G + 1], pb[:], AF.Identity,
                    scale=1.0 / 6.0, bias=cfin[0:H, 0:1],
                )

            # ---- main pairwise loop (triangular) ----
            def finalize(psum_t, bcol_ap, c, row0):
                o = opool.tile([H, CW], BF16, tag="o", name=f"o_{row0}_{c}")
                nc.scalar.activation(
                    o[:], psum_t[:], AF.Relu, scale=1.0 / 6.0, bias=bcol_ap
                )
                nc.vector.tensor_scalar_min(o[:], o[:], 1.0)
                nc.sync.dma_start(
                    out_d[row0 : row0 + 64, c * CW : (c + 1) * CW], o[:]
                )

            def main_group(G, mid_emit=()):
                W = N_NODES - CW * G
                psum_grp = {
                    c: psm.tile([H, CW], F32, tag="psm", name=f"psg_{G}_{c}")
                    for c in range(G, NCHUNK)
                }
                started = {c: False for c in range(G, NCHUNK)}
                mid = dict(mid_emit)
                for u in range(16):  # pair batches of 2
                    if u in mid:
                        mid[u]()
                    # diagonal trim: later pairs sit past the start of their
                    # diagonal chunk; those outputs are below the local
                    # diagonal and discarded by the host mirror. The trim
                    # grows monotonically with u, so earlier (wider) pairs
                    # initialize every psum region before narrower ones land.
                    half = 32 * u
                    Wu = W - half
                    src0 = CW * G + half
                    neg = u in M_ACT  # m2 holds -m on the ACT route
                    m2 = mp.tile([D, 2 * N_NODES], FP8, tag="m2")
                    e2 = ep.tile([D, 2 * N_NODES], BF16, tag="e2")
                    for q in range(2):
                        t = 32 * G + 2 * u + q
                        if neg:
                            # -ratio*m = relu(-ratio*(g_j + g_i + b2a))
                            nc.scalar.activation(
                                m2[:, q * Wu : (q + 1) * Wu],
                                g2bm[:, src0:N_NODES],
                                AF.Relu,
                                scale=-1.0,
                                bias=g2smn[:, 16 * t : 16 * t + 1],
                            )
                        else:
                            nc.vector.tensor_scalar(
                                m2[:, q * Wu : (q + 1) * Wu],
                                g2bm[:, src0:N_NODES],
                                g2sm[:, 16 * t : 16 * t + 1],
                                0.0,
                                OP.add,
                                OP.min,
                            )
                        # e' = min(P_i * P_j, al*eratio)  (independent of m)
                        nc.vector.tensor_scalar(
                            e2[:, q * Wu : (q + 1) * Wu],
                            P2[:, src0:N_NODES],
                            Ps[:, 16 * t : 16 * t + 1],
                            alrat[:, 0:1],
                            OP.mult,
                            OP.min,
                        )

                    # e-part: bf16 matmul per pair (precision-critical);
                    # m-part: one fp8 DoubleRow matmul for both q's, moving
                    # [128, 2, wdt], psum rows 4u..4u+3. Matmuls are grouped
                    # by stationary (all chunks back-to-back) so redundant
                    # weight loads can be elided.
                    m2i = m2[:, 0 : 2 * Wu].rearrange("p (i n) -> p i n", i=2)

                    def spans(c):
                        if c == G:
                            return 0, CW - half, half
                        return (c - G) * CW - half, CW, 0

                    for q in range(2):
                        s = 2 * u + q
                        for c in range(G, NCHUNK):
                            pt = psum_grp[c]
                            n0, wdt, p0 = spans(c)
                            nc.tensor.matmul(
                                pt[:, p0 : p0 + wdt],
                                w32e[:, H * s : H * (s + 1)],
                                e2[:, q * Wu + n0 : q * Wu + n0 + wdt],
                                start=(not started[c]),
                                stop=False,
                                skip_group_check=True,
                            )
                            started[c] = True
                    for c in range(G, NCHUNK):
                        pt = psum_grp[c]
                        n0, wdt, p0 = spans(c)
                        nc.tensor.matmul(
                            pt[:, p0 : p0 + wdt],
                            w32m_s[:, u, :, :],
                            m2i[:, :, n0 : n0 + wdt],
                            start=False,
                            stop=False,
                            perf_mode=DR,
                            skip_group_check=True,
                        )
                # rank-1 c_j add closes each (G, chunk); then finalize
                bcol_ap = Bcol[:, G : G + 1]
                for c in range(G, NCHUNK):
                    pt = psum_grp[c]
                    nc.tensor.matmul(
                        pt[:],
                        ones64[:],
                        c_row[0:1, c * CW : (c + 1) * CW],
                        start=False,
                        stop=True,
                        skip_group_check=True,
                    )
                    finalize(pt, bcol_ap, c, 64 * G)

            # interleaved emission: each engine's stream is its schedule, so
            # group G=c's pairwise work is emitted right after preamble
            # chunk c; the next chunk's preamble is emitted mid-group in two
            # stages so its ACT/DVE ops don't starve the pairwise feed
            def preamble_chunk(c):
                preamble_part1(c)
                preamble_part2(c)

            preamble_chunk(2)
            main_group(2, mid_emit={6: lambda: preamble_chunk(1)})
            main_group(1, mid_emit={6: lambda: preamble_chunk(0)})
            main_group(0)

    nc.compile()
    return nc


def _host_inputs(x, W_enc, b_enc, W1a, b1a, W1b, b1b, W2a, b2a, W2b, b2b):
    """Build the per-core input maps (core c gets x rolled by -c)."""
    BF = ml_dtypes.bfloat16
    F8 = ml_dtypes.float8_e4m3fn
    w = W2b[0].astype(np.float64)
    K0 = float(w @ b2a.astype(np.float64))
    SW = float(w.sum())
    # K0 cancels: c_row/Bcol are computed from g + b2a/2, adding LAM*K0
    CONST = -LAM * AL * SW + float(b2b[0])

    # e-part stationaries [128, 32 slots, 64 out-rows] bf16: slot s = pair,
    # psum rows 2s (top half) / 2s+1 (bottom)
    w32e = np.zeros((D, 32, H), np.float32)
    for s in range(32):
        w32e[0:H, s, 2 * s] = (LAM * w).astype(np.float32)
        w32e[H:D, s, 2 * s + 1] = (LAM * w).astype(np.float32)
    w32e = w32e.reshape(D, 32 * H)
    # m-part DoubleRow stationaries [128, 16 slots, 2 interleave(=q), 64]
    # slot u, interleave q -> psum rows 4u+2q (top half) and 4u+2q+1 (bottom)
    w32m = np.zeros((D, 16, 2, H), np.float32)
    for u in range(16):
        for q in range(2):
            sgn = 1.0 if u in M_ACT else -1.0
            w32m[0:H, u, q, 4 * u + 2 * q] = (sgn * LAM * w).astype(np.float32)
            w32m[H:D, u, q, 4 * u + 2 * q + 1] = (sgn * LAM * w).astype(
                np.float32)
    w32m = w32m.reshape(D, 16 * 2 * H)

    W_combo = (W1a.astype(np.float64) @ W_enc.astype(np.float64))  # [H, RAW]
    b_combo = (W1a.astype(np.float64) @ b_enc.astype(np.float64)
               + b1a.astype(np.float64))  # [H]

    # selu-constant folding: each selu returns selu(x)+lam*al; compensate in
    # the next layer's bias / the g-copy bias
    dlt1 = LAM * AL * W1b.astype(np.float64).sum(axis=1)   # [D]
    dltg = LAM * AL * W2a.astype(np.float64).sum(axis=1)   # [H]

    # wpack [128, 4H + D + H]: 4 k-slices of W_combo.T, then W1b.T (rows
    # 0:64), then W2a.T
    wpack = np.zeros((D, 4 * H + D + H), np.float32)
    WcT = np.ascontiguousarray(W_combo.T, np.float32)  # [512, 64]
    for k in range(4):
        wpack[:, k * H : (k + 1) * H] = WcT[k * D : (k + 1) * D, :]
    wpack[0:H, 4 * H : 4 * H + D] = W1b.T
    wpack[:, 4 * H + D :] = W2a.T
    wpack = wpack.astype(BF)

    # stationary-weight quantization compensation: ratio = w/fp8(w), folded
    # into the m-operands (scale) and P's bias/clamp
    wq = (LAM * w).astype(F8).astype(np.float64)
    ratio = np.where(wq != 0, (LAM * w) / np.where(wq == 0, 1, wq), 1.0)
    ratio2 = np.concatenate([ratio, ratio])

    b1b_eff = b1b.astype(np.float64) - dlt1
    cpack = np.zeros((D, 10), np.float32)
    cpack[0:H, 0] = b_combo
    cpack[0:H, 1] = LAM * b_combo
    cpack[:, 2] = b1b_eff
    cpack[:, 3] = LAM * b1b_eff
    cpack[0:H, 4] = 0.5 * b2a - dltg          # gbias: g2c = g + b2a/2
    cpack[:, 5] = ratio2
    cpack[:, 6] = CONST / 6.0 + 0.5
    cpack[0:H, 8] = 0.5 * np.log(AL)          # P bias (bf16 e-stationary:
    cpack[:, 9] = AL                          # no ratio folding needed)

    common = {
        "wpack": wpack,
        "cpack": cpack,
        "wlb": (LAM * w).reshape(H, 1).astype(BF),
        "w32e": w32e.astype(BF),
        "w32m": w32m.astype(F8),
        "ones64": np.ones((1, H), np.float32).astype(BF),
    }
    in_maps = []
    for c in range(NCORES):
        m = dict(common)
        m["xT"] = np.ascontiguousarray(np.roll(x, -c, axis=0).T).astype(BF)
        in_maps.append(m)
    return in_maps


def _assemble(results):
    """Mirror per-core upper-triangle bands into the full symmetric output."""
    O = np.zeros((N_NODES, N_NODES), np.float32)
    for c in range(NCORES):
        U = np.roll(np.asarray(results[c]["out"]).astype(np.float32), c, axis=1)
        O[c::8, :] = U  # rows c, c+8, ... (192 rows in order)
    Ou = np.triu(O)
    return (Ou + Ou.T - np.diag(np.diag(Ou))).astype(np.float32)


def kernel(x, W_enc, b_enc, W1a, b1a, W1b, b1b, W2a, b2a, W2b, b2b):
    from concourse.bass_utils import run_bass_kernel_spmd

    global _compiled
    if _compiled is None:
        _compiled = _build_program()
    in_maps = _host_inputs(
        np.asarray(x, np.float32),
        np.asarray(W_enc, np.float32), np.asarray(b_enc, np.float32),
        np.asarray(W1a, np.float32), np.asarray(b1a, np.float32),
        np.asarray(W1b, np.float32), np.asarray(b1b, np.float32),
        np.asarray(W2a, np.float32), np.asarray(b2a, np.float32),
        np.asarray(W2b, np.float32), np.asarray(b2b, np.float32),
    )
    res = run_bass_kernel_spmd(_compiled, in_maps, list(range(NCORES)))
    return _assemble(res.results)


# revision 91
# speedup vs baseline: 1.0340x; 1.0340x over previous
"""Trainium2 Bass kernel for nn_LCAMatrixModel (pairwise selu-MLP scoring).

o[i,j] = hardsigmoid( sum_h W2b[h]*selu(g[i,h]+g[j,h]+b2a[h]) + b2b )
with g = f(x) a small per-node MLP chain. o is symmetric.

Decomposition (u = g_i+g_j+b2a, m = min(u,0)):
  sum_h w*selu(u) = lam*(c_i+c_j+K0) - lam*al*SW
                    + sum_h (lam*w)*al*e^m + sum_h (-lam*w)*m
with c_i = sum_h w*g[i,h] rank-1 (PE close via ones64 x c_row + Bcol bias).
Only the e-part (al*e^m) and m-part need the full N^2*H elementwise work:
  m-part: min(u,0) scaled by ratio = w/fp8(w) (folds the fp8 stationary
          quantization into the data), written as fp8; reduced by ONE fp8
          DoubleRow matmul per (pair-batch u, chunk) covering 4 output rows
          (2 pairs x 2 rows) at 0.5 cycles/row. 12 of 16 u's compute -m on
          ACT (relu(-u), sign folded into the stationary), 4 on DVE.
  e-part: al*e^m = min(P_i*P_j, al) with P = sqrt(al)*e^(g+b2a/2), computed
          on DVE (tensor_scalar mult+min, per-pair P_i scalar read straight
          from a col-shifted f32 tile - no gather DMAs); kept bf16 and
          reduced by per-pair bf16 matmuls (fp8 here fails the error gate:
          both its saturated and mid-range quantization alone exceed it).
Other tricks: encoder folded into nn1 on the host (W1a@W_enc); selu's
-lam*al constant folded into downstream biases (one fused DVE
scalar_tensor_tensor per selu); pairs trimmed below the local diagonal with
a monotone per-u trim (early pairs initialize all psum regions); preamble
chunks emitted in order 2,1,0 interleaved into the pairwise groups so each
engine's instruction stream (= its schedule) overlaps preamble and main
work; matmuls grouped by stationary so weight reloads can be elided.

Sharding: np.roll(x, -c) per core -> core c owns global rows {c, c+8, ...};
each core computes its local upper triangle (output is symmetric); host
mirrors. Output DMA'd as bf16, host converts to f32.
"""
import sys

sys.path.insert(0, "/opt/trn_rl_repo")

import numpy as np
import ml_dtypes

N_NODES = 1536
RAW = 512
D = 128
H = 64
NCORES = 8
ROWS = N_NODES // NCORES  # own rows per core = 192
PAIRS = ROWS // 2         # 96
GROUPS = PAIRS // 32      # 3 groups of 32 pairs (64 out rows each)
NCHUNK = 3                # 512-wide j chunks
CW = 512

LAM = 1.0507009873554805
AL = 1.6732632423543772

# ---- route tables (per u in 0..15, same for each group), tuned against
# probed HW rates: DVE ts dual-op ~0.74 ns/col, ACT ~1.23, Pool useless,
# PE fp8 DoubleRow ~0.4 ns/col + ldweights ----
# m-op engine: ACT computes -m = relu(-u) (so downstream flips signs).
# e' comes from the DVE P-product route for every u (exp of fp8 m would
# amplify quantization); ACT's only main-loop work is these m ops.
M_ACT = {4, 5, 6, 7, 8, 9, 10, 11, 12, 13, 14, 15}

_compiled = None


def _build_program():
    import concourse.bacc as bacc
    import concourse.mybir as mybir
    import concourse.tile as tile

    F32 = mybir.dt.float32
    BF16 = mybir.dt.bfloat16
    FP8 = mybir.dt.float8e4
    AF = mybir.ActivationFunctionType
    OP = mybir.AluOpType
    DR = mybir.MatmulPerfMode.DoubleRow

    nc = bacc.Bacc("TRN2", target_bir_lowering=False, debug=False)

    # ---- DRAM I/O ----
    xT_d = nc.dram_tensor("xT", [RAW, N_NODES], BF16, kind="ExternalInput")
    # wpack: wcT (4 k-slices of (W1a@W_enc).T), w1bT (rows 0:64), w2aT  [128, 448]
    wpack_d = nc.dram_tensor("wpack", [D, 4 * H + D + H], BF16, kind="ExternalInput")
    wlb_d = nc.dram_tensor("wlb", [H, 1], BF16, kind="ExternalInput")
    # cpack columns: 0 bc, 1 bcl, 2 b1b, 3 b1bl, 4 gbias, 5 ratio2,
    # 6 cfin, 7 ratio2n, 8 pbias, 9 alrat
    cpack_d = nc.dram_tensor("cpack", [D, 10], F32, kind="ExternalInput")
    # e-part stationaries: per pair-slot [128, 64] bf16, 32 slots
    w32e_d = nc.dram_tensor("w32e", [D, 32 * H], BF16, kind="ExternalInput")
    # m-part DoubleRow stationaries: per u-slot [128, 2, 64] fp8, 16 slots
    w32m_d = nc.dram_tensor("w32m", [D, 16 * 2 * H], FP8, kind="ExternalInput")
    ones64_d = nc.dram_tensor("ones64", [1, H], BF16, kind="ExternalInput")
    out_d = nc.dram_tensor("out", [ROWS, N_NODES], BF16, kind="ExternalOutput")

    with tile.TileContext(nc) as tc:
        with (
            tc.tile_pool(name="cst", bufs=1) as cst,
            tc.tile_pool(name="pre", bufs=2) as pre,
            tc.tile_pool(name="mp", bufs=8) as mp,
            tc.tile_pool(name="ep", bufs=8) as ep,
            tc.tile_pool(name="op", bufs=4) as opool,
            tc.tile_pool(name="ps", bufs=2, space="PSUM") as ps,
            tc.tile_pool(name="psm", bufs=6, space="PSUM") as psm,
        ):
            # ---- load constants (scalar + gpsimd queues; sync queue is
            # reserved for the x chunks so compute can start ASAP) ----
            wpack = cst.tile([D, 4 * H + D + H], BF16)
            nc.scalar.dma_start(wpack[:], wpack_d[:])
            wlb = cst.tile([H, 1], BF16)
            nc.scalar.dma_start(wlb[:], wlb_d[:])
            wcT = wpack[:, 0 : 4 * H]
            w1bT = wpack[0:H, 4 * H : 4 * H + D]
            w2aT = wpack[:, 4 * H + D : 4 * H + D + H]
            cpack = cst.tile([D, 10], F32)
            nc.scalar.dma_start(cpack[:], cpack_d[:])
            bc = cpack[0:H, 0:1]
            bcl = cpack[0:H, 1:2]
            b1b = cpack[:, 2:3]
            b1bl = cpack[:, 3:4]
            gbias = cpack[0:H, 4:5]
            ratio2 = cpack[:, 5:6]
            cfin = cpack[:, 6:7]
            pbias = cpack[0:H, 8:9]
            alrat = cpack[:, 9:10]
            ones64 = cst.tile([1, H], BF16)
            nc.scalar.dma_start(ones64[:], ones64_d[:])
            w32e = cst.tile([D, 32 * H], BF16)
            nc.gpsimd.dma_start(w32e[:], w32e_d[:])
            w32m = cst.tile([D, 16 * 2 * H], FP8)
            nc.gpsimd.dma_start(w32m[:], w32m_d[:])
            w32m_s = w32m[:, :].rearrange("p (s i m) -> p s i m", i=2, m=H)

            F32R = mybir.dt.float32r

            # ---- selu chain helper: out = selu(psum + b) + lam*al ----
            # (the +lam*al constant is folded into the next layer's bias)
            def selu_from_psum(out_ap, pa, b_raw, b_lam, p):
                r = pre.tile([p, CW], F32, tag="selr")
                nc.scalar.activation(r[:], pa, AF.Relu, bias=b_lam, scale=LAM)
                m = pre.tile([p, CW], F32, tag="selm")
                nc.vector.tensor_scalar(m[:], pa, b_raw, 0.0, OP.add, OP.min)
                e = pre.tile([p, CW], F32, tag="sele")
                nc.scalar.activation(e[:], m[:], AF.Exp)
                nc.vector.scalar_tensor_tensor(
                    out_ap, e[:], LAM * AL, r[:], OP.mult, OP.add
                )

            # ---- per-chunk preamble (chunk order 2,1,0 so group G=2 can
            # start its pairwise work while chunks 1,0 are still cooking) ----
            xt = cst.tile([D, 4 * N_NODES], BF16)
            a1T = cst.tile([H, N_NODES], BF16)
            hT = cst.tile([D, N_NODES], BF16)
            # g2c = g + b2a/2 - delta_g (selu-const compensation), f32 [64, N]
            g2c = cst.tile([H, N_NODES], F32)
            g2b = cst.tile([D, N_NODES], BF16)  # both halves = g2c (bf16)
            # per-pair scalar sources: col 16t = [g2c_i1 ; g2c_i2] (bottom
            # half shifted by 8 cols)
            g2s = cst.tile([D, N_NODES], F32)
            # ratio-scaled m-operands (w-quantization folded into the data):
            # g2bm = ratio*g2c both halves, g2sm/g2smn = +-ratio*g2s
            g2bm = cst.tile([D, N_NODES], BF16)
            g2sm = cst.tile([D, N_NODES], F32)
            g2smn = cst.tile([D, N_NODES], F32)
            # P-route tensors: P = sqrt(al*eratio)*exp(g2c); P2 bf16 both
            # halves, Ps f32 scalar source (bottom shifted by 8, like g2s)
            P2 = cst.tile([D, N_NODES], BF16)
            Ps = cst.tile([D, N_NODES], F32)
            gown = cst.tile([H, ROWS], BF16)
            c_row = cst.tile([1, N_NODES], BF16)
            Bcol = cst.tile([H, GROUPS], F32)

            g2b_own = g2b[0:H, :].rearrange("p (a b) -> p a b", b=8)

            # the shifted bottom halves leave the last 8 cols unwritten;
            # they are never read as scalars but full-width copies of them are
            nc.vector.memset(g2s[H:D, N_NODES - 8 : N_NODES], 0.0)

            def preamble_part1(c):
                cs = slice(c * CW, (c + 1) * CW)
                for k in range(4):
                    nc.sync.dma_start(
                        xt[:, k * N_NODES + c * CW : k * N_NODES + (c + 1) * CW],
                        xT_d[k * D : (k + 1) * D, cs],
                    )
                # a1 = selu(W_combo @ x + b_combo)   (encoder folded in)
                pa = ps.tile([H, CW], F32, tag="ps")
                for k in range(4):
                    nc.tensor.matmul(
                        pa[:],
                        wcT[:, k * H : (k + 1) * H],
                        xt[:, k * N_NODES + c * CW : k * N_NODES + (c + 1) * CW],
                        start=(k == 0),
                        stop=(k == 3),
                    )
                selu_from_psum(a1T[:, cs], pa[:], bc[:, 0:1], bcl[:, 0:1], H)

            def preamble_part2(c):
                cs = slice(c * CW, (c + 1) * CW)
                # h = selu(W1b @ a1 + b1b)
                ph = ps.tile([D, CW], F32, tag="ps")
                nc.tensor.matmul(
                    ph[:], w1bT[:], a1T[:, cs], start=True, stop=True,
                )
                selu_from_psum(hT[:, cs], ph[:], b1b[:, 0:1], b1bl[:, 0:1], D)
                # g = W2a @ h
                pg = ps.tile([H, CW], F32, tag="ps")
                nc.tensor.matmul(
                    pg[:], w2aT[:], hT[:, cs], start=True, stop=True,
                )
                # g2c = g + b2a/2 - delta_g (bias-compensated), from psum
                nc.scalar.activation(g2c[:, cs], pg[:], AF.Identity, bias=gbias)
                # g2b both halves = g2c (bf16); DVE casts the top, the idle
                # SP/DMA path replicates bottom halves (same dtype, no cast)
                nc.vector.tensor_copy(g2b[0:H, cs], g2c[:, cs])
                nc.sync.dma_start(g2b[H:D, cs], g2b[0:H, cs])
                # scalar-source tiles: g2s top = g2c, bottom = g2c shifted by
                # 8 cols (so col 16t = [g_i1; g_i2] for pair t).
                # Chunk order 2,1,0 means cols (c+1)*CW..+8 already exist.
                nc.vector.tensor_copy(g2s[0:H, cs], g2c[:, cs])
                wsh = CW if c < NCHUNK - 1 else CW - 8
                nc.sync.dma_start(
                    g2s[H:D, c * CW : c * CW + wsh],
                    g2c[:, c * CW + 8 : c * CW + 8 + wsh],
                )
                # ratio-scaled m-operand tiles
                nc.vector.tensor_scalar(
                    g2bm[0:H, cs], g2c[:, cs], ratio2[0:H, 0:1], None, OP.mult
                )
                nc.sync.dma_start(g2bm[H:D, cs], g2bm[0:H, cs])
                nc.vector.tensor_scalar(
                    g2sm[:, cs], g2s[:, cs], ratio2[:, 0:1], None, OP.mult
                )
                nc.scalar.activation(
                    g2smn[:, cs], g2sm[:, cs], AF.Identity, scale=-1.0
                )
                # P-route tensors: Ps top f32 = sqrt(al*eratio)*exp(g2c),
                # bottom shifted by 8; P2 = Ps in bf16 (both halves aligned)
                nc.scalar.activation(
                    Ps[0:H, cs], g2c[:, cs], AF.Exp, bias=pbias
                )
                nc.vector.tensor_copy(P2[0:H, cs], Ps[0:H, cs])
                nc.sync.dma_start(P2[H:D, cs], P2[0:H, cs])
                nc.sync.dma_start(
                    Ps[H:D, c * CW : c * CW + wsh],
                    Ps[0:H, c * CW + 8 : c * CW + 8 + wsh],
                )
                # c_row chunk = wl.T @ g2b (includes lam; K0/2 shift folded
                # into CONST on the host)
                pc = ps.tile([1, CW], F32, tag="ps")
                nc.tensor.matmul(
                    pc[:], wlb[:], g2b[0:H, cs], start=True, stop=True,
                )
                nc.scalar.activation(c_row[0:1, cs], pc[:], AF.Copy)
                # group-G own-col gather (for Bcol only; off critical path)
                G = c
                osl = slice(H * G, H * (G + 1))
                nc.sync.dma_start(gown[:, osl], g2b_own[:, osl, 0:1])
                pb = ps.tile([H, 1], F32, tag="ps", name=f"pb_{G}")
                nc.tensor.matmul(
                    pb[:], gown[:, osl], wlb[:], start=True, stop=True,
                )
                nc.scalar.activation(
                    Bcol[:, G : G + 1], pb[:], AF.Identity,
                    scale=1.0 / 6.0, bias=cfin[0:H, 0:1],
                )

            # ---- main pairwise loop (triangular) ----
            def finalize(psum_t, bcol_ap, c, row0):
                o = opool.tile([H, CW], BF16, tag="o", name=f"o_{row0}_{c}")
                nc.scalar.activation(
                    o[:], psum_t[:], AF.Relu, scale=1.0 / 6.0, bias=bcol_ap
                )
                nc.vector.tensor_scalar_min(o[:], o[:], 1.0)
                nc.sync.dma_start(
                    out_d[row0 : row0 + 64, c * CW : (c + 1) * CW], o[:]
                )

            def main_group(G, mid_emit=()):
                W = N_NODES - CW * G
                psum_grp = {
                    c: psm.tile([H, CW], F32, tag="psm", name=f"psg_{G}_{c}")
                    for c in range(G, NCHUNK)
                }
                started = {c: False for c in range(G, NCHUNK)}
                mid = dict(mid_emit)
                for u in range(16):  # pair batches of 2
                    if u in mid:
                        mid[u]()
                    # diagonal trim: later pairs sit past the start of their
                    # diagonal chunk; those outputs are below the local
                    # diagonal and discarded by the host mirror. The trim
                    # grows monotonically with u, so earlier (wider) pairs
                    # initialize every psum region before narrower ones land.
                    half = 32 * u
                    Wu = W - half
                    src0 = CW * G + half
                    neg = u in M_ACT  # m2 holds -m on the ACT route
                    m2 = mp.tile([D, 2 * N_NODES], FP8, tag="m2")
                    e2 = ep.tile([D, 2 * N_NODES], BF16, tag="e2")
                    for q in range(2):
                        t = 32 * G + 2 * u + q
                        if neg:
                            # -ratio*m = relu(-ratio*(g_j + g_i + b2a))
                            nc.scalar.activation(
                                m2[:, q * Wu : (q + 1) * Wu],
                                g2bm[:, src0:N_NODES],
                                AF.Relu,
                                scale=-1.0,
                                bias=g2smn[:, 16 * t : 16 * t + 1],
                            )
                        else:
                            nc.vector.tensor_scalar(
                                m2[:, q * Wu : (q + 1) * Wu],
                                g2bm[:, src0:N_NODES],
                                g2sm[:, 16 * t : 16 * t + 1],
                                0.0,
                                OP.add,
                                OP.min,
                            )
                        # e' = min(P_i * P_j, al*eratio)  (independent of m)
                        nc.vector.tensor_scalar(
                            e2[:, q * Wu : (q + 1) * Wu],
                            P2[:, src0:N_NODES],
                            Ps[:, 16 * t : 16 * t + 1],
                            alrat[:, 0:1],
                            OP.mult,
                            OP.min,
                        )

                    # e-part: bf16 matmul per pair (precision-critical);
                    # m-part: one fp8 DoubleRow matmul for both q's, moving
                    # [128, 2, wdt], psum rows 4u..4u+3. Matmuls are grouped
                    # by stationary (all chunks back-to-back) so redundant
                    # weight loads can be elided.
                    m2i = m2[:, 0 : 2 * Wu].rearrange("p (i n) -> p i n", i=2)

                    def spans(c):
                        if c == G:
                            return 0, CW - half, half
                        return (c - G) * CW - half, CW, 0

                    for q in range(2):
                        s = 2 * u + q
                        for c in range(G, NCHUNK):
                            pt = psum_grp[c]
                            n0, wdt, p0 = spans(c)
                            nc.tensor.matmul(
                                pt[:, p0 : p0 + wdt],
                                w32e[:, H * s : H * (s + 1)],
                                e2[:, q * Wu + n0 : q * Wu + n0 + wdt],
                                start=(not started[c]),
                                stop=False,
                                skip_group_check=True,
                            )
                            started[c] = True
                    for c in range(G, NCHUNK):
                        pt = psum_grp[c]
                        n0, wdt, p0 = spans(c)
                        nc.tensor.matmul(
                            pt[:, p0 : p0 + wdt],
                            w32m_s[:, u, :, :],
                            m2i[:, :, n0 : n0 + wdt],
                            start=False,
                            stop=False,
                            perf_mode=DR,
                            skip_group_check=True,
                        )
                # rank-1 c_j add closes each (G, chunk); then finalize
                bcol_ap = Bcol[:, G : G + 1]
                for c in range(G, NCHUNK):
                    pt = psum_grp[c]
                    nc.tensor.matmul(
                        pt[:],
                        ones64[:],
                        c_row[0:1, c * CW : (c + 1) * CW],
                        start=False,
                        stop=True,
                        skip_group_check=True,
                    )
                    finalize(pt, bcol_ap, c, 64 * G)

            # interleaved emission: each engine's stream is its schedule, so
            # group G=c's pairwise work is emitted right after preamble
            # chunk c; the next chunk's preamble is emitted mid-group in two
            # stages so its ACT/DVE ops don't starve the pairwise feed
            def preamble_chunk(c):
                preamble_part1(c)
                preamble_part2(c)

            preamble_chunk(2)
            main_group(2, mid_emit={4: lambda: preamble_chunk(1)})
            main_group(1, mid_emit={4: lambda: preamble_chunk(0)})
            main_group(0)

    nc.compile()
    return nc


def _host_inputs(x, W_enc, b_enc, W1a, b1a, W1b, b1b, W2a, b2a, W2b, b2b):
    """Build the per-core input maps (core c gets x rolled by -c)."""
    BF = ml_dtypes.bfloat16
    F8 = ml_dtypes.float8_e4m3fn
    w = W2b[0].astype(np.float64)
    K0 = float(w @ b2a.astype(np.float64))
    SW = float(w.sum())
    # K0 cancels: c_row/Bcol are computed from g + b2a/2, adding LAM*K0
    CONST = -LAM * AL * SW + float(b2b[0])

    # e-part stationaries [128, 32 slots, 64 out-rows] bf16: slot s = pair,
    # psum rows 2s (top half) / 2s+1 (bottom)
    w32e = np.zeros((D, 32, H), np.float32)
    for s in range(32):
        w32e[0:H, s, 2 * s] = (LAM * w).astype(np.float32)
        w32e[H:D, s, 2 * s + 1] = (LAM * w).astype(np.float32)
    w32e = w32e.reshape(D, 32 * H)
    # m-part DoubleRow stationaries [128, 16 slots, 2 interleave(=q), 64]
    # slot u, interleave q -> psum rows 4u+2q (top half) and 4u+2q+1 (bottom)
    w32m = np.zeros((D, 16, 2, H), np.float32)
    for u in range(16):
        for q in range(2):
            sgn = 1.0 if u in M_ACT else -1.0
            w32m[0:H, u, q, 4 * u + 2 * q] = (sgn * LAM * w).astype(np.float32)
            w32m[H:D, u, q, 4 * u + 2 * q + 1] = (sgn * LAM * w).astype(
                np.float32)
    w32m = w32m.reshape(D, 16 * 2 * H)

    W_combo = (W1a.astype(np.float64) @ W_enc.astype(np.float64))  # [H, RAW]
    b_combo = (W1a.astype(np.float64) @ b_enc.astype(np.float64)
               + b1a.astype(np.float64))  # [H]

    # selu-constant folding: each selu returns selu(x)+lam*al; compensate in
    # the next layer's bias / the g-copy bias
    dlt1 = LAM * AL * W1b.astype(np.float64).sum(axis=1)   # [D]
    dltg = LAM * AL * W2a.astype(np.float64).sum(axis=1)   # [H]

    # wpack [128, 4H + D + H]: 4 k-slices of W_combo.T, then W1b.T (rows
    # 0:64), then W2a.T
    wpack = np.zeros((D, 4 * H + D + H), np.float32)
    WcT = np.ascontiguousarray(W_combo.T, np.float32)  # [512, 64]
    for k in range(4):
        wpack[:, k * H : (k + 1) * H] = WcT[k * D : (k + 1) * D, :]
    wpack[0:H, 4 * H : 4 * H + D] = W1b.T
    wpack[:, 4 * H + D :] = W2a.T
    wpack = wpack.astype(BF)

    # stationary-weight quantization compensation: ratio = w/fp8(w), folded
    # into the m-operands (scale) and P's bias/clamp
    wq = (LAM * w).astype(F8).astype(np.float64)
    ratio = np.where(wq != 0, (LAM * w) / np.where(wq == 0, 1, wq), 1.0)
    ratio2 = np.concatenate([ratio, ratio])

    b1b_eff = b1b.astype(np.float64) - dlt1
    cpack = np.zeros((D, 10), np.float32)
    cpack[0:H, 0] = b_combo
    cpack[0:H, 1] = LAM * b_combo
    cpack[:, 2] = b1b_eff
    cpack[:, 3] = LAM * b1b_eff
    cpack[0:H, 4] = 0.5 * b2a - dltg          # gbias: g2c = g + b2a/2
    cpack[:, 5] = ratio2
    cpack[:, 6] = CONST / 6.0 + 0.5
    cpack[0:H, 8] = 0.5 * np.log(AL)          # P bias (bf16 e-stationary:
    cpack[:, 9] = AL                          # no ratio folding needed)

    common = {
        "wpack": wpack,
        "cpack": cpack,
        "wlb": (LAM * w).reshape(H, 1).astype(BF),
        "w32e": w32e.astype(BF),
        "w32m": w32m.astype(F8),
        "ones64": np.ones((1, H), np.float32).astype(BF),
    }
    in_maps = []
    for c in range(NCORES):
        m = dict(common)
        m["xT"] = np.ascontiguousarray(np.roll(x, -c, axis=0).T).astype(BF)
        in_maps.append(m)
    return in_maps


def _assemble(results):
    """Mirror per-core upper-triangle bands into the full symmetric output."""
    O = np.zeros((N_NODES, N_NODES), np.float32)
    for c in range(NCORES):
        U = np.roll(np.asarray(results[c]["out"]).astype(np.float32), c, axis=1)
        O[c::8, :] = U  # rows c, c+8, ... (192 rows in order)
    Ou = np.triu(O)
    return (Ou + Ou.T - np.diag(np.diag(Ou))).astype(np.float32)


def kernel(x, W_enc, b_enc, W1a, b1a, W1b, b1b, W2a, b2a, W2b, b2b):
    from concourse.bass_utils import run_bass_kernel_spmd

    global _compiled
    if _compiled is None:
        _compiled = _build_program()
    in_maps = _host_inputs(
        np.asarray(x, np.float32),
        np.asarray(W_enc, np.float32), np.asarray(b_enc, np.float32),
        np.asarray(W1a, np.float32), np.asarray(b1a, np.float32),
        np.asarray(W1b, np.float32), np.asarray(b1b, np.float32),
        np.asarray(W2a, np.float32), np.asarray(b2a, np.float32),
        np.asarray(W2b, np.float32), np.asarray(b2b, np.float32),
    )
    res = run_bass_kernel_spmd(_compiled, in_maps, list(range(NCORES)))
    return _assemble(res.results)
